# revision 9
# baseline (speedup 1.0000x reference)
"""Trainium2 Bass kernel for nn_CA_1580547973147 (class-token attention block).

Reference computation (per batch b):
    qkv = x @ qkv_w.T + qkv_b                  # only class-token query used
    q0  = qkv[:, 0, 0]     (= x[:,0] @ Wq.T + bq)
    k   = x @ Wk.T + bk ;  v = x @ Wv.T + bv
    attn = softmax(SCALE * q0_h . k_h)         # [H, N] per batch
    cls  = (attn @ v) @ proj_w.T + proj_b      # [1, C]
    out  = concat([cls, x[:, 1:]], axis=1)

Algebraic restructuring (per batch):
    scores[h, n] = sum_c g[h, c] * x[n, c]      with g = blockdiag(q0+bq) @ Wk
      (the bk term is constant per row h and cancels in softmax)
    cls[c'] = sum_c z[h(c'), c] * Wv[c', c] + bv[c']   with z = attn @ x
      (sum(attn) == 1 so bv passes through exactly)
so K and V are never materialized.

Implementation notes:
  - everything over HBM is fp8 e4m3 (x in both layouts, all weights);
    PSUM accumulation is fp32, on-chip intermediates bf16. gt carries a
    x8 gain folded back out of the exp scale.
  - the 8 local batches run as 2 groups of 4 via PE column-tiling:
    batch j of a group owns array column group j (tile_position=
    (0,32j), PSUM rows 32j..32j+12), so four M=12 matmuls stream
    concurrently. Interleaved accumulation in one PSUM bank uses a
    single start/stop bracket per bank: start clears the whole bank's
    has_written bits, then each element's first write overwrites
    (per-element bits); the banks are pre-zeroed by DVE memsets so the
    never-written lanes stay finite.
  - softmax (one exp ACT with accumulate), the e/z transposes, and the
    copy-outs are batched across the whole 128-partition group.
  - cls2/c2t run per group so group 0's tail work fills the PE idle gap
    while group 1's x is still streaming; the final proj matmul is
    accumulation-split across two PE column groups.
  - DMA order: consts/wq/wk, group-0 x, tail weights, group-1 x with
    the n-major tiles split per n-tile so z(g1) starts per-tile.
  - a gpsimd-memset warm-up matmul chain holds the PE HAM clock gate
    open from t~0 so the real work runs at 2.4 GHz.

Sharding: pure data-parallel over batch, 8 batches per core on 8 cores.
Rows 1..N-1 of the output equal x, assembled on the host.
"""

import numpy as np
import ml_dtypes
from contextlib import ExitStack

import concourse.bass as bass
import concourse.mybir as mybir
import concourse.tile as tile
from concourse import bacc
from concourse import bass_utils

F32 = mybir.dt.float32
BF16 = mybir.dt.bfloat16
FP8 = mybir.dt.float8e4
EXP = mybir.ActivationFunctionType.Exp
IDENT = mybir.ActivationFunctionType.Identity
ADD = mybir.AluOpType.add

B, N, C, H = 64, 577, 768, 12
D = C // H
SCALE = D ** -0.5
NCORES = 8
BB = B // NCORES          # local batches per core
CT = C // 128             # 6 c-tiles
NT0 = N // 128            # 4 full n-tiles
NREM = N - NT0 * 128      # 65
NT = NT0 + 1              # 5 n-tiles
BH = BB * H               # 96 (b, h) pairs per core
NP2 = 578                 # x_t columns padded even
NG = 2                    # batch groups per core
GB = 4                    # batches per group (one per PE column group)
GAIN = 8.0                # fp8 gain on gt, folded out of the exp scale
NWARM = 32                # warm-up matmuls (~9.5us of PE activity)

np8 = ml_dtypes.float8_e4m3
npb = ml_dtypes.bfloat16


def build_program():
    nc = bacc.Bacc("TRN2", target_bir_lowering=False, debug=False)

    # x_tg[g, p, t, j, n] = x[4g+j, n, 128t+p]   (c-major tiles)
    # x_ng[g, p, t, j, c] = x[4g+j, 128t+p, c]   (n-major tiles)
    # x_n4g[g, p, j, c]   = x[4g+j, 512+p, c]    (n remainder)
    x_tg = nc.dram_tensor("x_tg", [NG, 128, CT, GB, NP2], FP8,
                          kind="ExternalInput").ap()
    x_ng = nc.dram_tensor("x_ng", [NG, 128, NT0, GB, C], FP8,
                          kind="ExternalInput").ap()
    x_n4g = nc.dram_tensor("x_n4g", [NG, NREM, GB, C], FP8,
                           kind="ExternalInput").ap()
    wqk = nc.dram_tensor("wqk", [128, 2, CT, C], FP8, kind="ExternalInput").ap()
    wv_t = nc.dram_tensor("wv_t", [128, CT, C], FP8, kind="ExternalInput").ap()
    proj_t = nc.dram_tensor("proj_t", [128, CT, C], FP8,
                            kind="ExternalInput").ap()
    idb = nc.dram_tensor("idb", [128, 128], BF16, kind="ExternalInput").ap()
    cstf = nc.dram_tensor("cstf", [128, 12], F32, kind="ExternalInput").ap()
    pb_b = nc.dram_tensor("pb_b", [BB, C], F32, kind="ExternalInput").ap()
    # qp0 zeros ++ x0t pre-tiled, one fp8 blob
    qpx = nc.dram_tensor("qpx", [128, CT * BH + CT * BB], FP8,
                         kind="ExternalInput").ap()
    out0 = nc.dram_tensor("out0", [BB, C], F32, kind="ExternalOutput").ap()

    CH = [(0, 512), (512, C)]  # free-dim chunks of C (psum bank bounded)

    with tile.TileContext(nc) as tc, ExitStack() as ctx:
        singles = ctx.enter_context(tc.tile_pool(name="singles", bufs=1))
        xtp = ctx.enter_context(tc.tile_pool(name="xtp", bufs=2))
        xnp = ctx.enter_context(tc.tile_pool(name="xnp", bufs=2))
        ep = ctx.enter_context(tc.tile_pool(name="ep", bufs=2))
        etp = ctx.enter_context(tc.tile_pool(name="etp", bufs=2))
        zsp = ctx.enter_context(tc.tile_pool(name="zsp", bufs=2))
        sm = ctx.enter_context(tc.tile_pool(name="sm", bufs=8))
        # PSUM banks: pbig 2x[128,1024]f32 = 4; warm 1; et/zt bf16 1 each = 7
        pbig = ctx.enter_context(tc.tile_pool(name="pbig", bufs=2, space="PSUM"))
        ptp = ctx.enter_context(tc.tile_pool(name="ptp", bufs=1, space="PSUM"))

        # ---- DMA stream (issue order = arrival order) ----
        qpx_sb = singles.tile([128, CT * BH + CT * BB], FP8)
        nc.sync.dma_start(out=qpx_sb, in_=qpx)
        qp_sb = qpx_sb[:, :CT * BH].rearrange("p (t bh) -> p t bh", bh=BH)
        x0_sb = qpx_sb[:, CT * BH:].rearrange("p (t b) -> p t b", b=BB)
        id_sb = singles.tile([128, 128], BF16)
        nc.sync.dma_start(out=id_sb, in_=idb)
        cst_sb = singles.tile([128, 12], F32)
        nc.sync.dma_start(out=cst_sb, in_=cstf)
        bq_sb = cst_sb[:, 0:6]
        bv_sb = cst_sb[:, 6:12]
        wqk_sb = singles.tile([128, 2, CT, C], FP8)
        nc.sync.dma_start(out=wqk_sb[:, 0], in_=wqk[:, 0])
        nc.sync.dma_start(out=wqk_sb[:, 1], in_=wqk[:, 1])
        wq_sb = wqk_sb[:, 0]
        wk_sb = wqk_sb[:, 1]

        # x order: both groups' c-major tiles first (scores g0/g1 are at
        # the front of the PE queue), then the n-major tiles split per
        # n-tile so z starts as each tile lands, group 1's last.
        xt_tiles = []
        xn_tiles = []
        for g in range(NG):
            xt_g = xtp.tile([128, CT, GB, NP2], FP8, tag="xt", name=f"xt_g{g}")
            nc.sync.dma_start(out=xt_g, in_=x_tg[g])
            xt_tiles.append(xt_g)
        xn_g0 = xnp.tile([128, NT, GB, C], FP8, tag="xn", name="xn_g0")
        for t in range(NT0):
            nc.sync.dma_start(out=xn_g0[:, t], in_=x_ng[0][:, t])
        nc.sync.dma_start(out=xn_g0[:NREM, NT0], in_=x_n4g[0])
        xn_tiles.append(xn_g0)
        # tail weights arrive mid-stream for the group-0 tail work
        wv_sb = singles.tile([128, CT, C], FP8)
        nc.sync.dma_start(out=wv_sb, in_=wv_t)
        pj_sb = singles.tile([128, CT, C], FP8)
        nc.sync.dma_start(out=pj_sb, in_=proj_t)
        pb_sb = singles.tile([BB, C], F32)
        nc.sync.dma_start(out=pb_sb, in_=pb_b)
        xn_g1 = xnp.tile([128, NT, GB, C], FP8, tag="xn", name="xn_g1")
        for t in range(NT0):
            nc.sync.dma_start(out=xn_g1[:, t], in_=x_ng[1][:, t])
        nc.sync.dma_start(out=xn_g1[:NREM, NT0], in_=x_n4g[1])
        xn_tiles.append(xn_g1)

        # ---- PE warm-up: accumulate zeros so the HAM clock gate opens
        # before the real weights arrive (no DMA dependency) ----
        warm_sb = singles.tile([128, 512], FP8)
        nc.gpsimd.memset(warm_sb, 0.0)
        warm_ps = ptp.tile([128, 512], F32, tag="warm", name="warm_ps")
        for i in range(NWARM):
            nc.tensor.matmul(warm_ps, warm_sb[:, 0:128], warm_sb,
                             start=(i == 0), stop=(i == NWARM - 1))

        # ---- q0 = x0 @ Wq.T -> [BB, C] ----
        q0_ps = pbig.tile([128, 1024], F32, tag="big", name="q0_ps")
        for c0, c1 in CH:
            for t in range(CT):
                nc.tensor.matmul(
                    q0_ps[:BB, c0:c1], x0_sb[:, t, :], wq_sb[:, t, c0:c1],
                    start=(t == 0), stop=(t == CT - 1))
        q0_sb = singles.tile([BB, C], BF16)
        nc.vector.tensor_copy(out=q0_sb, in_=q0_ps[:BB, :C])

        # ---- Q' block-diag [C, BH]: Q'[64h+d, 12b+h] = q0[b, 64h+d] + bq ----
        q0t_ps = ptp.tile([128, 768], BF16, tag="zt", name="q0t_ps")
        for t in range(CT):
            nc.tensor.transpose(q0t_ps[:, t * BB:(t + 1) * BB],
                                q0_sb[:, t * 128:(t + 1) * 128], id_sb[:BB, :BB])
        for t in range(CT):
            for half in range(2):
                h0 = 2 * t + half
                p0 = 64 * half
                nc.scalar.activation(
                    out=qp_sb[p0:p0 + 64, t, h0::12],
                    in_=q0t_ps[p0:p0 + 64, t * BB:(t + 1) * BB],
                    func=IDENT, bias=bq_sb[p0:p0 + 64, t:t + 1], scale=1.0)

        # ---- g = Q'.T @ Wk -> [BH, C] ; gt = GAIN * g.T [C, BH] fp8 ----
        g_ps = pbig.tile([128, 1024], F32, tag="big", name="g_ps")
        for c0, c1 in CH:
            for t in range(CT):
                nc.tensor.matmul(
                    g_ps[:BH, c0:c1], qp_sb[:, t, :], wk_sb[:, t, c0:c1],
                    start=(t == 0), stop=(t == CT - 1))
        g_sb = singles.tile([BH, C], BF16)
        nc.vector.tensor_copy(out=g_sb, in_=g_ps[:BH, :C])
        gt_ps = ptp.tile([128, 768], BF16, tag="et", name="gt_ps")
        for t in range(CT):
            nc.tensor.transpose(gt_ps[:, 128 * t:128 * t + BH],
                                g_sb[:, t * 128:(t + 1) * 128], id_sb[:BH, :BH])
        gt_sb = singles.tile([128, CT, BH], FP8)
        nc.scalar.mul(
            out=gt_sb,
            in_=gt_ps.rearrange("p (t x) -> p t x", x=128)[:, :, :BH],
            mul=GAIN)

        zt_sb = singles.tile([128, CT, BH], FP8)
        clst_sb = singles.tile([128, CT, BB], FP8)

        def emit_scores(g):
            # s[32j+h, n] = sum_c GAIN * g[12(4g+j)+h, c] x[4g+j, n, c]
            xt_g = xt_tiles[g]
            s_ps = pbig.tile([128, 1024], F32, tag="big", name=f"s_ps{g}")
            nc.vector.memset(s_ps[:, 0:NP2], 0.0)
            for t in range(CT):
                for j in range(GB):
                    nc.tensor.matmul(
                        s_ps[32 * j:32 * j + H, 0:512],
                        gt_sb[:, t, 12 * (GB * g + j):12 * (GB * g + j) + 12],
                        xt_g[:, t, j, 0:512],
                        start=(t == 0 and j == 0),
                        stop=(t == CT - 1 and j == GB - 1),
                        tile_position=(0, 32 * j), skip_group_check=True)
            for t in range(CT):
                for j in range(GB):
                    nc.tensor.matmul(
                        s_ps[32 * j:32 * j + H, 512:512 + 66],
                        gt_sb[:, t, 12 * (GB * g + j):12 * (GB * g + j) + 12],
                        xt_g[:, t, j, 512:NP2],
                        start=(t == 0 and j == 0),
                        stop=(t == CT - 1 and j == GB - 1),
                        tile_position=(0, 32 * j), skip_group_check=True)
            return s_ps

        def emit_softmax(g, s_ps):
            # e = exp(SCALE/GAIN * s); pad col -> 1.0, subtracted below
            e_sb = ep.tile([128, NP2], BF16, tag="e", name=f"e{g}")
            dd = sm.tile([128, 1], F32, tag="st", name=f"d_{g}")
            nc.scalar.activation(out=e_sb, in_=s_ps[:, 0:NP2],
                                 func=EXP, bias=0.0, scale=SCALE / GAIN,
                                 accum_out=dd)
            rec = sm.tile([128, 1], F32, tag="st", name=f"rec{g}")
            nc.vector.tensor_scalar(rec, dd, -1.0, None, ADD)
            nc.vector.reciprocal(rec, rec)
            return e_sb, rec

        def emit_et(g, e_sb):
            # eT [n, 32j+h] per n-tile, all 4 batches in one transpose
            et_ps = ptp.tile([128, 640], BF16, tag="et", name=f"et_ps{g}")
            for t in range(NT):
                w = 128 if t < NT0 else 66  # includes the pad column as row 65
                nc.tensor.transpose(et_ps[:w, 128 * t:128 * t + 128],
                                    e_sb[:, 128 * t:128 * t + w],
                                    id_sb[:128, :128])
            et_sb = etp.tile([128, NT, 128], FP8, tag="et", name=f"et{g}")
            nc.vector.tensor_copy(
                out=et_sb[:, :NT0, :],
                in_=et_ps[:, :512].rearrange("p (t n) -> p t n", n=128))
            nc.vector.tensor_copy(out=et_sb[:NREM, NT0, :],
                                  in_=et_ps[:NREM, 512:640])
            return et_sb

        def emit_z(g, et_sb, rec):
            # z[32j+h, c] = sum_n e[32j+h, n] x[4g+j, n, c], then * rec
            xn_g = xn_tiles[g]
            z_ps = pbig.tile([128, 1024], F32, tag="big", name=f"z_ps{g}")
            nc.vector.memset(z_ps[:, 0:C], 0.0)
            for t in range(NT):
                w = 128 if t < NT0 else NREM
                for c0, c1 in CH:
                    for j in range(GB):
                        nc.tensor.matmul(
                            z_ps[32 * j:32 * j + H, c0:c1],
                            et_sb[:w, t, 32 * j:32 * j + H],
                            xn_g[:w, t, j, c0:c1],
                            start=(t == 0 and j == 0),
                            stop=(t == NT - 1 and j == GB - 1),
                            tile_position=(0, 32 * j), skip_group_check=True)
            z_sb = zsp.tile([128, C], BF16, tag="z", name=f"z{g}")
            # normalization split across scalar+vector to cut the latency
            nc.scalar.activation(out=z_sb[:, 0:512], in_=z_ps[:, 0:512],
                                 func=IDENT, bias=0.0, scale=rec)
            nc.vector.tensor_scalar_mul(z_sb[:, 512:C], z_ps[:, 512:C], rec)
            return z_sb

        def emit_zt(g, z_sb):
            # zt [c, 12b+h]: 6 batched transposes, strided copy-out to fp8
            zt_ps = ptp.tile([128, 768], BF16, tag="zt", name=f"zt_ps{g}")
            for t in range(CT):
                nc.tensor.transpose(zt_ps[:, 128 * t:128 * (t + 1)],
                                    z_sb[:, 128 * t:128 * (t + 1)],
                                    id_sb[:128, :128])
            src = zt_ps.rearrange("p (t j h) -> p t j h", j=GB, h=32)
            dst = zt_sb[:, :, 12 * GB * g:12 * GB * (g + 1)].rearrange(
                "p t (j h) -> p t j h", h=12)
            for t in range(CT):
                nc.scalar.copy(out=dst[:, t], in_=src[:, t, :, :12])

        def emit_cls2(g):
            # cls2[12j+h, c'] = sum_c zt[c, 12(4g+j)+h] Wv[c', c]  (M=48)
            cls2_ps = pbig.tile([128, 1024], F32, tag="big", name=f"cls2_{g}")
            for c0, c1 in CH:
                for t in range(CT):
                    nc.tensor.matmul(
                        cls2_ps[:48, c0:c1],
                        zt_sb[:, t, 48 * g:48 * (g + 1)], wv_sb[:, t, c0:c1],
                        start=(t == 0), stop=(t == CT - 1))
            cls2_sb = zsp.tile([48, C], BF16, tag="cls2", name=f"cls2sb{g}")
            nc.vector.tensor_copy(out=cls2_sb, in_=cls2_ps[:48, :C])
            # c2t + diag-select (head h(c') = c'//64) + bv -> clst cols 4g..
            c2t_ps = ptp.tile([128, 768], BF16, tag="zt", name=f"c2t_{g}")
            for t in range(CT):
                nc.tensor.transpose(c2t_ps[:, 128 * t:128 * t + 48],
                                    cls2_sb[:, t * 128:(t + 1) * 128],
                                    id_sb[:48, :48])
            for t in range(CT):
                for half in range(2):
                    h0 = 2 * t + half
                    p0 = 64 * half
                    nc.scalar.activation(
                        out=clst_sb[p0:p0 + 64, t, GB * g:GB * (g + 1)],
                        in_=c2t_ps[p0:p0 + 64, 128 * t + h0:128 * t + 48:12],
                        func=IDENT, bias=bv_sb[p0:p0 + 64, t:t + 1], scale=1.0)

        # ---- pipelined group loop (all DMAs already queued above) ----
        s0 = emit_scores(0)
        e0, rec0 = emit_softmax(0, s0)
        s1 = emit_scores(1)
        e1, rec1 = emit_softmax(1, s1)
        et0 = emit_et(0, e0)
        z0 = emit_z(0, et0, rec0)
        et1 = emit_et(1, e1)
        emit_zt(0, z0)
        emit_cls2(0)           # fills the PE gap while group-1 x streams
        z1 = emit_z(1, et1, rec1)
        emit_zt(1, z1)
        emit_cls2(1)

        # ---- out0[b, c2] = sum_c' clst[c', b] proj[c2, c'] + pb ----
        # accumulation-split: even c'-tiles -> rows 0:8 (col group 0),
        # odd c'-tiles -> rows 64:72 (col group 2), summed on DVE.
        o_ps = pbig.tile([128, 1024], F32, tag="big", name="o_ps")
        nc.vector.memset(o_ps[:, 0:C], 0.0)
        for c0, c1 in CH:
            for t in range(CT):
                p0 = 64 * (t % 2)
                nc.tensor.matmul(
                    o_ps[p0:p0 + BB, c0:c1], clst_sb[:, t, :],
                    pj_sb[:, t, c0:c1],
                    start=(t == 0), stop=(t == CT - 1),
                    tile_position=(0, p0), skip_group_check=True)
        o_sb = singles.tile([BB, C], F32)
        nc.vector.tensor_tensor(o_sb, o_ps[0:BB, :C], pb_sb, ADD)
        nc.vector.tensor_tensor(o_sb, o_ps[64:64 + BB, :C], o_sb, ADD)
        nc.sync.dma_start(out=out0, in_=o_sb)

    nc.compile()
    return nc


_CACHED = None


def _get_program():
    global _CACHED
    if _CACHED is None:
        _CACHED = build_program()
    return _CACHED


def make_in_maps(x, qkv_w, qkv_b, proj_w, proj_b):
    x = np.ascontiguousarray(np.asarray(x, dtype=np.float32))
    qkv_w = np.asarray(qkv_w, dtype=np.float32)
    qkv_b = np.asarray(qkv_b, dtype=np.float32)
    proj_w = np.asarray(proj_w, dtype=np.float32)
    proj_b = np.asarray(proj_b, dtype=np.float32)

    def pretile(a):
        # [C, C] row-major -> [p, t, c] with row = 128 t + p
        return np.ascontiguousarray(
            a.reshape(CT, 128, C).transpose(1, 0, 2)).astype(np8)

    cstf = np.zeros((128, 12), np.float32)
    cstf[:, 0:6] = qkv_b[0:C].reshape(CT, 128).T
    cstf[:, 6:12] = qkv_b[2 * C:3 * C].reshape(CT, 128).T
    shared = {
        "wqk": np.ascontiguousarray(np.stack(
            [pretile(qkv_w[0:C].T), pretile(qkv_w[C:2 * C])], axis=1)),
        "wv_t": pretile(qkv_w[2 * C:3 * C].T),
        "proj_t": pretile(proj_w.T),
        "pb_b": np.ascontiguousarray(np.tile(proj_b, (BB, 1))),
        "cstf": cstf,
        "idb": np.eye(128, dtype=npb),
    }
    in_maps = []
    for c in range(NCORES):
        xb = x[c * BB:(c + 1) * BB]
        x8 = xb.astype(np8)
        m = dict(shared)
        # x_tg[g, p, t, j, n] = x[4g+j, n, 128t+p]
        xt = np.zeros((NG, 128, CT, GB, NP2), np8)
        xt[:, :, :, :, :N] = x8.transpose(2, 0, 1).reshape(
            CT, 128, NG, GB, N).transpose(2, 1, 0, 3, 4)
        m["x_tg"] = xt
        # x_ng[g, p, t, j, c] = x[4g+j, 128t+p, c]
        m["x_ng"] = np.ascontiguousarray(
            x8[:, :NT0 * 128].reshape(NG, GB, NT0, 128, C).transpose(
                0, 3, 2, 1, 4))
        # x_n4g[g, p, j, c] = x[4g+j, 512+p, c]
        m["x_n4g"] = np.ascontiguousarray(
            x8[:, NT0 * 128:].reshape(NG, GB, NREM, C).transpose(0, 2, 1, 3))
        qpx = np.zeros((128, CT * BH + CT * BB), np8)
        qpx[:, CT * BH:] = xb[:, 0, :].T.reshape(CT, 128, BB).transpose(
            1, 0, 2).reshape(128, CT * BB).astype(np8)
        m["qpx"] = qpx
        in_maps.append(m)
    return in_maps


def kernel(x, qkv_w, qkv_b, proj_w, proj_b, _trace=False):
    nc = _get_program()
    in_maps = make_in_maps(x, qkv_w, qkv_b, proj_w, proj_b)
    res = bass_utils.run_bass_kernel_spmd(
        nc, in_maps, core_ids=list(range(NCORES)), trace=_trace)
    out = np.array(x, dtype=np.float32, copy=True)
    for c in range(NCORES):
        out[c * BB:(c + 1) * BB, 0, :] = res.results[c]["out0"]
    kernel._last_results = res
    return out


# revision 22
# speedup vs baseline: 1.1832x; 1.1832x over previous
"""Trainium2 Bass kernel for nn_CA_1580547973147 (class-token attention block).

Reference computation (per batch b):
    qkv = x @ qkv_w.T + qkv_b                  # only class-token query used
    q0  = qkv[:, 0, 0]     (= x[:,0] @ Wq.T + bq)
    k   = x @ Wk.T + bk ;  v = x @ Wv.T + bv
    attn = softmax(SCALE * q0_h . k_h)         # [H, N] per batch
    cls  = (attn @ v) @ proj_w.T + proj_b      # [1, C]
    out  = concat([cls, x[:, 1:]], axis=1)

Algebraic restructuring (per batch):
    scores[h, n] = sum_c g[h, c] * x[n, c]      with g = blockdiag(q0+bq) @ Wk
      (bk is constant per row h and cancels in softmax)
    cls[c'] = sum_c z[h(c'), c] * Wv[c', c] + bv[c']   with z = attn @ x
      (sum(attn) == 1 so bv passes through exactly)
so K and V are never materialized. Bias folding on the host:
    g = blockdiag(q0) @ Wk + gb        with gb[h,:] = bq[64h:64h+64] @ Wk-block
    out = clst @ proj.T + pb_eff       with pb_eff = proj_b + proj_w @ bv

Implementation notes:
  - everything over HBM is fp8 e4m3; PSUM is fp32, on-chip intermediates
    bf16. The q'/g path carries a x8 gain folded out of the exp scale so
    fp8 quantization error stays small.
  - the 8 local batches run as 2 groups of 4 via PE column-tiling:
    batch j of a group owns array column group j (tile_position=
    (0,32j), PSUM rows 32j..32j+12), so four M=12 matmuls stream
    concurrently. Interleaved accumulation in one PSUM bank uses a
    single start/stop bracket per bank (start clears the whole bank's
    has_written; each element's first write then overwrites), with the
    banks pre-zeroed by DVE memsets so never-written lanes stay finite.
  - transpose-and-pack steps are identity-selection matmuls: streaming
    only the wanted identity columns yields eT / zT / diag-selected cls
    already packed, so each needs just one contiguous copy-out instead
    of a chain of small strided scalar ops (which previously paced the
    whole tail through the scalar engine queue).
  - cls2/c2t run per group so group 0's tail work fills the PE idle gap
    while group 1's x streams; the final proj matmul is accumulation-
    split across two PE column groups; dummy filler matmuls bridge the
    two unavoidable PE waits so the HAM clock gate never drops to 1/2.
  - x DMA is split per 128-tile (more parallel DMA queues) with group
    1's n-major tiles last, so z(g1) starts per-tile as data lands.

Sharding: pure data-parallel over batch, 8 batches per core on 8 cores.
Rows 1..N-1 of the output equal x, assembled on the host.
"""

import numpy as np
import ml_dtypes
from contextlib import ExitStack

import concourse.bass as bass
import concourse.mybir as mybir
import concourse.tile as tile
from concourse import bacc
from concourse import bass_utils

F32 = mybir.dt.float32
BF16 = mybir.dt.bfloat16
FP8 = mybir.dt.float8e4
EXP = mybir.ActivationFunctionType.Exp
IDENT = mybir.ActivationFunctionType.Identity
ADD = mybir.AluOpType.add

B, N, C, H = 64, 577, 768, 12
D = C // H
SCALE = D ** -0.5
NCORES = 8
BB = B // NCORES          # local batches per core
CT = C // 128             # 6 c-tiles
NT0 = N // 128            # 4 full n-tiles
NREM = N - NT0 * 128      # 65
NT = NT0 + 1              # 5 n-tiles
BH = BB * H               # 96 (b, h) pairs per core
NP2 = 578                 # x_t columns padded even
NG = 2                    # batch groups per core
GB = 4                    # batches per group (one per PE column group)
GH = GB * H               # 48
GAIN = 8.0                # fp8 gain on the q'/g path, folded out of exp
NWARM = 16

np8 = ml_dtypes.float8_e4m3
npb = ml_dtypes.bfloat16


def build_program():
    nc = bacc.Bacc("TRN2", target_bir_lowering=False, debug=False)

    # x_tg[g, p, t, j, n] = x[4g+j, n, 128t+p]   (c-major tiles)
    # x_ng[g, p, t, j, c] = x[4g+j, 128t+p, c]   (n-major tiles)
    # x_n4g[g, p, j, c]   = x[4g+j, 512+p, c]    (n remainder)
    x_tg = nc.dram_tensor("x_tg", [NG, 128, CT, GB, NP2], FP8,
                          kind="ExternalInput").ap()
    x_ng = nc.dram_tensor("x_ng", [NG, 128, NT0, GB, C], FP8,
                          kind="ExternalInput").ap()
    x_n4g = nc.dram_tensor("x_n4g", [NG, NREM, GB, C], FP8,
                           kind="ExternalInput").ap()
    wqk = nc.dram_tensor("wqk", [128, 2, CT, C], FP8, kind="ExternalInput").ap()
    wv_t = nc.dram_tensor("wv_t", [128, CT, C], FP8, kind="ExternalInput").ap()
    proj_t = nc.dram_tensor("proj_t", [128, CT, C], FP8,
                            kind="ExternalInput").ap()
    idb = nc.dram_tensor("idb", [128, 128], BF16, kind="ExternalInput").ap()
    gb_d = nc.dram_tensor("gb_d", [128, CT, BH], BF16,
                          kind="ExternalInput").ap()
    pb_b = nc.dram_tensor("pb_b", [BB, C], F32, kind="ExternalInput").ap()
    # qp0 zeros ++ x0t pre-tiled, one fp8 blob
    qpx = nc.dram_tensor("qpx", [128, CT * BH + CT * BB], FP8,
                         kind="ExternalInput").ap()
    out0 = nc.dram_tensor("out0", [BB, C], F32, kind="ExternalOutput").ap()

    CH = [(0, 512), (512, C)]  # free-dim chunks of C (psum bank bounded)

    with tile.TileContext(nc) as tc, ExitStack() as ctx:
        singles = ctx.enter_context(tc.tile_pool(name="singles", bufs=1))
        xtp = ctx.enter_context(tc.tile_pool(name="xtp", bufs=2))
        xnp = ctx.enter_context(tc.tile_pool(name="xnp", bufs=2))
        ep = ctx.enter_context(tc.tile_pool(name="ep", bufs=2))
        etp = ctx.enter_context(tc.tile_pool(name="etp", bufs=2))
        zsp = ctx.enter_context(tc.tile_pool(name="zsp", bufs=2))
        sm = ctx.enter_context(tc.tile_pool(name="sm", bufs=8))
        # PSUM banks: pbig 2x2 + et(bufs2) 2 + zt 1 + c2t 1 = 8
        pbig = ctx.enter_context(tc.tile_pool(name="pbig", bufs=2, space="PSUM"))
        ptp = ctx.enter_context(tc.tile_pool(name="ptp", bufs=1, space="PSUM"))
        pte = ctx.enter_context(tc.tile_pool(name="pte", bufs=2, space="PSUM"))

        # ---- DMA stream (issue order = arrival order; fine-grained
        # starts spread across more hardware DMA queues) ----
        qpx_sb = singles.tile([128, CT * BH + CT * BB], FP8)
        nc.sync.dma_start(out=qpx_sb, in_=qpx)
        qp_sb = qpx_sb[:, :CT * BH]
        x0_sb = qpx_sb[:, CT * BH:].rearrange("p (t b) -> p t b", b=BB)
        id_sb = singles.tile([128, 128], BF16)
        nc.sync.dma_start(out=id_sb, in_=idb)
        gb_sb = singles.tile([128, CT, BH], BF16)
        nc.sync.dma_start(out=gb_sb, in_=gb_d)
        wqk_sb = singles.tile([128, 2, CT, C], FP8)
        for i in range(2):
            for hl in range(2):
                nc.sync.dma_start(out=wqk_sb[:, i, 3 * hl:3 * hl + 3],
                                  in_=wqk[:, i, 3 * hl:3 * hl + 3])
        wq_sb = wqk_sb[:, 0]
        wk_sb = wqk_sb[:, 1]
        # column selector: id columns {32j + h, h<12} -> packed 48
        idsel = id_sb.rearrange("p (j x) -> p j x", x=32)[:, :, :H]

        xt_tiles = []
        xn_tiles = []
        for g in range(NG):
            xt_g = xtp.tile([128, CT, GB, NP2], FP8, tag="xt", name=f"xt_g{g}")
            for t in range(CT):
                nc.sync.dma_start(out=xt_g[:, t], in_=x_tg[g][:, t])
            xt_tiles.append(xt_g)
        xn_g0 = xnp.tile([128, NT, GB, C], FP8, tag="xn", name="xn_g0")
        for t in range(NT0):
            nc.sync.dma_start(out=xn_g0[:, t], in_=x_ng[0][:, t])
        nc.sync.dma_start(out=xn_g0[:NREM, NT0], in_=x_n4g[0])
        xn_tiles.append(xn_g0)
        # tail weights arrive mid-stream for the group-0 tail work
        wv_sb = singles.tile([128, CT, C], FP8)
        pj_sb = singles.tile([128, CT, C], FP8)
        for w_sb, w_d in ((wv_sb, wv_t), (pj_sb, proj_t)):
            for hl in range(2):
                nc.sync.dma_start(out=w_sb[:, 3 * hl:3 * hl + 3],
                                  in_=w_d[:, 3 * hl:3 * hl + 3])
        pb_sb = singles.tile([BB, C], F32)
        nc.sync.dma_start(out=pb_sb, in_=pb_b)
        xn_g1 = xnp.tile([128, NT, GB, C], FP8, tag="xn", name="xn_g1")
        for t in range(NT0):
            nc.sync.dma_start(out=xn_g1[:, t], in_=x_ng[1][:, t])
        nc.sync.dma_start(out=xn_g1[:NREM, NT0], in_=x_n4g[1])
        xn_tiles.append(xn_g1)

        # ---- PE warm-up on the early tiny qpx blob: holds the HAM
        # clock gate open until the real weights arrive ----
        warm_sb = qpx_sb[:, 0:512]
        warm_ps = pte.tile([128, 512], F32, tag="et", name="warm_ps")
        for i in range(NWARM):
            nc.tensor.matmul(warm_ps, warm_sb[:, 0:128], warm_sb,
                             start=(i == 0), stop=(i == NWARM - 1))

        # ---- q0 = x0 @ Wq.T -> [BB, C] (bq folded into gb) ----
        q0_ps = pbig.tile([128, 1024], F32, tag="big", name="q0_ps")
        for c0, c1 in CH:
            for t in range(CT):
                nc.tensor.matmul(
                    q0_ps[:BB, c0:c1], x0_sb[:, t, :], wq_sb[:, t, c0:c1],
                    start=(t == 0), stop=(t == CT - 1))
        q0_sb = singles.tile([BB, C], BF16)
        nc.vector.tensor_copy(out=q0_sb, in_=q0_ps[:BB, :C])

        # ---- Q' block-diag [C, BH] with h-major columns (col = 8h+b):
        # Q'[128t+64hf+d, 8(2t+hf)+b] = q0[b, 128t+64hf+d], so each
        # transpose writes a contiguous aligned [64, 8] block ----
        q0t_big = pbig.tile([128, 1024], F32, tag="big", name="q0t_big")
        q0t_ps = q0t_big[:, :CT * BH]
        nc.vector.memset(q0t_ps, 0.0)
        for t in range(CT):
            for hf in range(2):
                b0 = 112 * t + 8 * hf
                nc.tensor.matmul(
                    q0t_ps[64 * hf:64 * hf + 64, b0:b0 + BB],
                    q0_sb[:, 128 * t + 64 * hf:128 * t + 64 * hf + 64],
                    id_sb[:BB, :BB],
                    start=True, stop=True,
                    tile_position=(0, 64 * hf), skip_group_check=True)
        nc.scalar.mul(out=qp_sb, in_=q0t_ps, mul=GAIN)

        # ---- g = GAIN * Q'.T @ Wk + gb -> gt [C, BH] fp8 ----
        qp_v = qp_sb.rearrange("p (t bh) -> p t bh", bh=BH)
        g_ps = pbig.tile([128, 1024], F32, tag="big", name="g_ps")
        for c0, c1 in CH:
            for t in range(CT):
                nc.tensor.matmul(
                    g_ps[:BH, c0:c1], qp_v[:, t, :], wk_sb[:, t, c0:c1],
                    start=(t == 0), stop=(t == CT - 1))
        g_sb = singles.tile([BH, C], BF16)
        nc.vector.tensor_copy(out=g_sb, in_=g_ps[:BH, :C])
        gt_ps = pte.tile([128, 768], BF16, tag="et", name="gt_ps")
        for t in range(CT):
            nc.tensor.transpose(gt_ps[:, 128 * t:128 * t + BH],
                                g_sb[:, t * 128:(t + 1) * 128], id_sb[:BH, :BH])
        gt_sb = singles.tile([128, CT, BH], FP8)
        nc.vector.tensor_tensor(
            gt_sb, gt_ps.rearrange("p (t x) -> p t x", x=128)[:, :, :BH],
            gb_sb, ADD)

        zt_sb = singles.tile([128, CT, BH], FP8)
        clst_sb = singles.tile([128, CT, BB], FP8)

        def emit_scores(g):
            # s[32j+h, n] = sum_c gt[c, 12(4g+j)+h] x[4g+j, n, c]
            xt_g = xt_tiles[g]
            s_ps = pbig.tile([128, 1024], F32, tag="big", name=f"s_ps{g}")
            nc.vector.memset(s_ps[:, 0:NP2], 0.0)
            for t in range(CT):
                for j in range(GB):
                    nc.tensor.matmul(
                        s_ps[32 * j:32 * j + H, 0:512],
                        gt_sb[:, t, GB * g + j::BB],
                        xt_g[:, t, j, 0:512],
                        start=(t == 0 and j == 0),
                        stop=(t == CT - 1 and j == GB - 1),
                        tile_position=(0, 32 * j), skip_group_check=True)
            for t in range(CT):
                for j in range(GB):
                    nc.tensor.matmul(
                        s_ps[32 * j:32 * j + H, 512:512 + 66],
                        gt_sb[:, t, GB * g + j::BB],
                        xt_g[:, t, j, 512:NP2],
                        start=(t == 0 and j == 0),
                        stop=(t == CT - 1 and j == GB - 1),
                        tile_position=(0, 32 * j), skip_group_check=True)
            return s_ps

        def emit_softmax(g, s_ps):
            # e = exp(SCALE/GAIN * s); pad col -> 1.0, subtracted below
            e_sb = ep.tile([128, NP2], BF16, tag="e", name=f"e{g}")
            dd = sm.tile([128, 1], F32, tag="st", name=f"d_{g}")
            nc.scalar.activation(out=e_sb, in_=s_ps[:, 0:NP2],
                                 func=EXP, bias=0.0, scale=SCALE / GAIN,
                                 accum_out=dd)
            rec = sm.tile([128, 1], F32, tag="st", name=f"rec{g}")
            nc.vector.tensor_scalar(rec, dd, -1.0, None, ADD)
            nc.vector.reciprocal(rec, rec)
            return e_sb, rec

        def emit_et(g, e_sb):
            # eT[n, 12j+h] per n-tile via identity-column selection
            et_ps = pte.tile([128, NT * GH], F32, tag="et", name=f"et_ps{g}")
            for t in range(NT):
                w = 128 if t < NT0 else 66  # includes the pad col as row 65
                nc.tensor.matmul(et_ps[:w, GH * t:GH * (t + 1)],
                                 e_sb[:, 128 * t:128 * t + w], idsel,
                                 start=True, stop=True)
            et_sb = etp.tile([128, NT, GH], FP8, tag="et", name=f"et{g}")
            nc.vector.tensor_copy(
                out=et_sb[:, :NT0, :],
                in_=et_ps[:, :NT0 * GH].rearrange("p (t n) -> p t n", n=GH))
            nc.scalar.copy(out=et_sb[:NREM, NT0, :],
                           in_=et_ps[:NREM, NT0 * GH:])
            return et_sb

        def emit_z(g, et_sb, rec):
            # z[32j+h, c] = sum_n e[32j+h, n] x[4g+j, n, c], then * rec
            xn_g = xn_tiles[g]
            z_ps = pbig.tile([128, 1024], F32, tag="big", name=f"z_ps{g}")
            nc.vector.memset(z_ps[:, 0:C], 0.0)
            for t in range(NT):
                w = 128 if t < NT0 else NREM
                for c0, c1 in CH:
                    for j in range(GB):
                        nc.tensor.matmul(
                            z_ps[32 * j:32 * j + H, c0:c1],
                            et_sb[:w, t, 12 * j:12 * j + 12],
                            xn_g[:w, t, j, c0:c1],
                            start=(t == 0 and j == 0),
                            stop=(t == NT - 1 and j == GB - 1),
                            tile_position=(0, 32 * j), skip_group_check=True)
            z_sb = zsp.tile([128, C], BF16, tag="z", name=f"z{g}")
            # normalization split across scalar+vector to cut the latency
            nc.scalar.activation(out=z_sb[:, 0:512], in_=z_ps[:, 0:512],
                                 func=IDENT, bias=0.0, scale=rec)
            nc.vector.tensor_scalar_mul(z_sb[:, 512:C], z_ps[:, 512:C], rec)
            return z_sb

        def emit_zt(g, z_sb, use_scalar):
            # zt[c, 12j+h] via identity-column selection, one copy-out
            zt_ps = ptp.tile([128, CT * GH], F32, tag="zt", name=f"zt_ps{g}")
            for t in range(CT):
                nc.tensor.matmul(zt_ps[:, GH * t:GH * (t + 1)],
                                 z_sb[:, 128 * t:128 * (t + 1)], idsel,
                                 start=True, stop=True)
            dst = zt_sb[:, :, GH * g:GH * (g + 1)]
            src = zt_ps.rearrange("p (t n) -> p t n", n=GH)
            if use_scalar:
                nc.scalar.copy(out=dst, in_=src)
            else:
                nc.vector.tensor_copy(out=dst, in_=src)

        def emit_cls2(g):
            # cls2[12j+h, c'] = sum_c zt[c, 12(4g+j)+h] Wv[c', c]  (M=48)
            cls2_ps = pbig.tile([128, 1024], F32, tag="big", name=f"cls2_{g}")
            for c0, c1 in CH:
                for t in range(CT):
                    nc.tensor.matmul(
                        cls2_ps[:GH, c0:c1],
                        zt_sb[:, t, GH * g:GH * (g + 1)], wv_sb[:, t, c0:c1],
                        start=(t == 0), stop=(t == CT - 1))
            cls2_sb = zsp.tile([GH, C], BF16, tag="cls2", name=f"cls2sb{g}")
            nc.vector.tensor_copy(out=cls2_sb, in_=cls2_ps[:GH, :C])
            # diag-select clst[128t+64hf+d, j] = cls2[12j+(2t+hf), .]
            # directly via per-half identity-column matmuls
            c2t_ps = pte.tile([128, 512], F32, tag="c2t", name=f"c2t_{g}",
                              bufs=1)
            for t in range(CT):
                for hf in range(2):
                    h0 = 2 * t + hf
                    nc.tensor.matmul(
                        c2t_ps[64 * hf:64 * hf + 64, GB * t:GB * (t + 1)],
                        cls2_sb[:, 128 * t + 64 * hf:128 * t + 64 * hf + 64],
                        id_sb[:GH, h0:GH:12],
                        start=True, stop=True)
            nc.scalar.copy(
                out=clst_sb[:, :, GB * g:GB * (g + 1)],
                in_=c2t_ps[:, :CT * GB].rearrange("p (t j) -> p t j", j=GB))

        def emit_fill(n, ncol, tag, name):
            f_ps = (ptp if tag == "zt" else pte).tile(
                [128, ncol], F32, tag=tag, name=name)
            for i in range(n):
                nc.tensor.matmul(f_ps[:, :ncol], warm_sb[:, 0:128],
                                 warm_sb[:, :ncol],
                                 start=(i == 0), stop=(i == n - 1))

        # ---- pipelined group loop (all DMAs already queued above) ----
        s0 = emit_scores(0)
        e0, rec0 = emit_softmax(0, s0)
        s1 = emit_scores(1)
        e1, rec1 = emit_softmax(1, s1)
        et0 = emit_et(0, e0)
        z0 = emit_z(0, et0, rec0)
        et1 = emit_et(1, e1)
        emit_zt(0, z0, True)
        emit_cls2(0)           # fills the PE gap while group-1 x streams
        z1 = emit_z(1, et1, rec1)
        emit_fill(8, 288, "zt", "fill1")   # bridge the zscale-g1 wait
        emit_zt(1, z1, False)
        emit_cls2(1)

        # ---- out0[b, c2] = sum_c' clst[c', b] proj[c2, c'] + pb_eff ----
        # accumulation-split: even c'-tiles -> rows 0:8 (col group 0),
        # odd -> rows 64:72 (col group 2), summed on DVE.
        o_ps = pbig.tile([128, 1024], F32, tag="big", name="o_ps")
        nc.vector.memset(o_ps[:, 0:C], 0.0)
        for c0, c1 in CH:
            for t in range(CT):
                p0 = 64 * (t % 2)
                nc.tensor.matmul(
                    o_ps[p0:p0 + BB, c0:c1], clst_sb[:, t, :],
                    pj_sb[:, t, c0:c1],
                    start=(t == 0), stop=(t == CT - 1),
                    tile_position=(0, p0), skip_group_check=True)
        o_sb = singles.tile([BB, C], F32)
        nc.vector.tensor_tensor(o_sb, o_ps[0:BB, :C], pb_sb, ADD)
        nc.vector.tensor_tensor(o_sb, o_ps[64:64 + BB, :C], o_sb, ADD)
        nc.sync.dma_start(out=out0, in_=o_sb)

    nc.compile()
    return nc


_CACHED = None


def _get_program():
    global _CACHED
    if _CACHED is None:
        _CACHED = build_program()
    return _CACHED


def make_in_maps(x, qkv_w, qkv_b, proj_w, proj_b):
    x = np.ascontiguousarray(np.asarray(x, dtype=np.float32))
    qkv_w = np.asarray(qkv_w, dtype=np.float32)
    qkv_b = np.asarray(qkv_b, dtype=np.float32)
    proj_w = np.asarray(proj_w, dtype=np.float32)
    proj_b = np.asarray(proj_b, dtype=np.float32)

    def pretile(a):
        # [C, C] row-major -> [p, t, c] with row = 128 t + p
        return np.ascontiguousarray(
            a.reshape(CT, 128, C).transpose(1, 0, 2)).astype(np8)

    bq = qkv_b[0:C]
    bv = qkv_b[2 * C:3 * C]
    wk = qkv_w[C:2 * C]
    # gb[h, c] = sum_d bq[64h+d] Wk[64h+d, c], tiled [p, t, 8h+b] * GAIN
    gb = np.einsum("hdc,hd->hc", wk.reshape(H, D, C), bq.reshape(H, D))
    gb_t = np.repeat(
        gb.T.reshape(CT, 128, H).transpose(1, 0, 2), BB, axis=2)
    shared = {
        "wqk": np.ascontiguousarray(np.stack(
            [pretile(qkv_w[0:C].T), pretile(wk)], axis=1)),
        "wv_t": pretile(qkv_w[2 * C:3 * C].T),
        "proj_t": pretile(proj_w.T),
        "pb_b": np.ascontiguousarray(
            np.tile(proj_b + proj_w @ bv, (BB, 1)).astype(np.float32)),
        "gb_d": (GAIN * gb_t).astype(npb),
        "idb": np.eye(128, dtype=npb),
    }
    in_maps = []
    for c in range(NCORES):
        xb = x[c * BB:(c + 1) * BB]
        x8 = xb.astype(np8)
        m = dict(shared)
        # x_tg[g, p, t, j, n] = x[4g+j, n, 128t+p]
        xt = np.zeros((NG, 128, CT, GB, NP2), np8)
        xt[:, :, :, :, :N] = x8.transpose(2, 0, 1).reshape(
            CT, 128, NG, GB, N).transpose(2, 1, 0, 3, 4)
        m["x_tg"] = xt
        # x_ng[g, p, t, j, c] = x[4g+j, 128t+p, c]
        m["x_ng"] = np.ascontiguousarray(
            x8[:, :NT0 * 128].reshape(NG, GB, NT0, 128, C).transpose(
                0, 3, 2, 1, 4))
        # x_n4g[g, p, j, c] = x[4g+j, 512+p, c]
        m["x_n4g"] = np.ascontiguousarray(
            x8[:, NT0 * 128:].reshape(NG, GB, NREM, C).transpose(0, 2, 1, 3))
        qpx = np.zeros((128, CT * BH + CT * BB), np8)
        qpx[:, CT * BH:] = xb[:, 0, :].T.reshape(CT, 128, BB).transpose(
            1, 0, 2).reshape(128, CT * BB).astype(np8)
        m["qpx"] = qpx
        in_maps.append(m)
    return in_maps


def kernel(x, qkv_w, qkv_b, proj_w, proj_b, _trace=False):
    nc = _get_program()
    in_maps = make_in_maps(x, qkv_w, qkv_b, proj_w, proj_b)
    res = bass_utils.run_bass_kernel_spmd(
        nc, in_maps, core_ids=list(range(NCORES)), trace=_trace)
    out = np.array(x, dtype=np.float32, copy=True)
    for c in range(NCORES):
        out[c * BB:(c + 1) * BB, 0, :] = res.results[c]["out0"]
    kernel._last_results = res
    return out
